# revision 1
# baseline (speedup 1.0000x reference)
"""MOLELinear (mixture-of-linear-experts) Trainium2 kernel.

Math (per group g): out_g = x_g @ (sum_e c[g,e] W_e + W_sh).T + (sum_e c[g,e] b_e + b_sh)

Sharding: data-parallel over the 32 groups -> 4 groups (8192 tokens) per core,
expert weights replicated. Host does layout-only prep (transposes / stacking, no
arithmetic); all FLOPs (weight mixing, bias mixing, GEMM, bias add) run on device.

Device plan per core:
  - DMA in: xT [512, 8192] (x shard transposed), WT [9, 512, 512] (transposed
    experts + shared), coefficient broadcast / bias tensors.
  - Mix weights on DVE: WmixT_g = sum_e c[g,e] WT_e + WT_sh via fused
    scalar_tensor_tensor FMAs (8 ops of [128, 2048] per group).
  - Mix biases on PE: tiny K=9 matmul per group.
  - Main GEMM on PE in float32r (1 cyc/row): psum[t128, o512] accumulates 4
    k-tiles plus a K=1 ones-row matmul that adds the mixed bias.
  - Drain PSUM->SBUF on ScalarE, DMA out.
"""
import ml_dtypes
import numpy as np

import concourse.bacc as bacc
import concourse.mybir as mybir
from concourse.alu_op_type import AluOpType
from concourse.tile import TileContext
from concourse.bass_utils import run_bass_kernel_spmd

N_CORES = 8
IN_F = 512
OUT_F = 512
N_EXPERTS = 8
N_GROUPS = 32
TOK_PER_GROUP = 2048
G_PER_CORE = N_GROUPS // N_CORES          # 4
TOK_PER_CORE = G_PER_CORE * TOK_PER_GROUP  # 8192
KT = IN_F // 128                           # 4 k-tiles
F32 = mybir.dt.float32
F32R = mybir.dt.float32r
BF16 = mybir.dt.bfloat16

_CACHE = {}


def _build():
    nc = bacc.Bacc(trn_type="TRN2")
    xT = nc.dram_tensor("xT", (IN_F, TOK_PER_CORE), F32, kind="ExternalInput")
    wt = nc.dram_tensor("wt", (N_EXPERTS + 1, IN_F, OUT_F), F32R, kind="ExternalInput")
    cb = nc.dram_tensor("cb", (128, G_PER_CORE * N_EXPERTS), F32, kind="ExternalInput")
    cx = nc.dram_tensor("cx", (N_EXPERTS + 1, G_PER_CORE), F32R, kind="ExternalInput")
    ball = nc.dram_tensor("ball", (N_EXPERTS + 1, OUT_F), F32R, kind="ExternalInput")
    ones = nc.dram_tensor("ones", (1, 128), BF16, kind="ExternalInput")
    out = nc.dram_tensor("out", (TOK_PER_CORE, OUT_F), F32, kind="ExternalOutput")

    with TileContext(nc) as tc:
        with (
            tc.tile_pool(name="wp", bufs=1) as wp,
            tc.tile_pool(name="mixp", bufs=1) as mixp,
            tc.tile_pool(name="smallp", bufs=1) as smallp,
            tc.tile_pool(name="xp", bufs=3) as xp,
            tc.tile_pool(name="op", bufs=3) as op,
            tc.tile_pool(name="psp", bufs=6, space="PSUM") as psp,
            tc.tile_pool(name="psb", bufs=2, space="PSUM") as psb,
        ):
            # ---- small DMAs first (cheap SP issues, unblock bias/mixing) ----
            cbt = smallp.tile([128, G_PER_CORE * N_EXPERTS], F32, tag="cb")
            nc.sync.dma_start(cbt[:], cb[:])
            cxt = smallp.tile([N_EXPERTS + 1, G_PER_CORE], F32R, tag="cx")
            nc.sync.dma_start(cxt[:], cx[:])
            ballt = smallp.tile([N_EXPERTS + 1, OUT_F], F32R, tag="ball")
            nc.sync.dma_start(ballt[:], ball[:])
            onest = smallp.tile([1, 128], BF16, tag="ones")
            nc.sync.dma_start(onest[:], ones[:])

            # ---- load all 9 experts' k-slice in ONE DMA per k-tile ----
            wt_r = wt[:].rearrange("e (kt p) o -> kt p e o", p=128)  # [4,128,9,512]
            wkt = []
            for kt in range(KT):
                t = wp.tile([128, (N_EXPERTS + 1) * OUT_F], F32R, tag=f"wkt{kt}")
                nc.sync.dma_start(
                    t[:].rearrange("p (e o) -> p e o", e=N_EXPERTS + 1), wt_r[kt]
                )
                wkt.append(t)
            wsb = {
                (e, kt): wkt[kt][:, e * OUT_F : (e + 1) * OUT_F]
                for e in range(N_EXPERTS + 1)
                for kt in range(KT)
            }

            # ---- mixed biases: mb_g = cx[:, g].T @ ball  (K=9, M=1, N=512) ----
            mbt = []
            for g in range(G_PER_CORE):
                pbg = psb.tile([1, OUT_F], F32, tag="pb")
                nc.tensor.matmul(pbg[:], cxt[:, g : g + 1], ballt[:], start=True, stop=True)
                mb = smallp.tile([1, OUT_F], BF16, tag=f"mb{g}")
                nc.vector.tensor_copy(mb[:], pbg[:])
                mbt.append(mb)

            # ---- mix weights on DVE: wmix_g = sum_e c[g,e]*WT_e + WT_sh ----
            # ---- mix per (group, k-tile); final FMA writes bf16 ----
            wmix = {}
            for g in range(G_PER_CORE):
                for kt in range(KT):
                    acc = mixp.tile([128, OUT_F], F32, tag="wma", bufs=2)
                    wm = mixp.tile([128, OUT_F], BF16, tag=f"wm{g}_{kt}")
                    nc.vector.scalar_tensor_tensor(
                        acc[:], wsb[(0, kt)],
                        cbt[:, g * N_EXPERTS : g * N_EXPERTS + 1],
                        wsb[(N_EXPERTS, kt)], AluOpType.mult, AluOpType.add,
                    )
                    for e in range(1, N_EXPERTS):
                        nc.vector.scalar_tensor_tensor(
                            acc[:] if e < N_EXPERTS - 1 else wm[:],
                            wsb[(e, kt)],
                            cbt[:, g * N_EXPERTS + e : g * N_EXPERTS + e + 1],
                            acc[:], AluOpType.mult, AluOpType.add,
                        )
                    wmix[(g, kt)] = wm

            # ---- main GEMM ----
            n_chunks = TOK_PER_CORE // 512  # 16 chunks of 512 tokens
            for ch in range(n_chunks):
                g = ch // (TOK_PER_GROUP // 512)
                t0 = ch * 512
                xs = xp.tile([128, KT * 512], F32, tag="x")
                nc.sync.dma_start(
                    xs[:].rearrange("p (kt t) -> p kt t", kt=KT),
                    xT[:, t0 : t0 + 512].rearrange("(kt p) t -> p kt t", p=128),
                )
                xb = xp.tile([128, KT * 512], BF16, tag="xb")
                nc.scalar.copy(xb[:], xs[:])
                oc = op.tile([128, 4 * OUT_F], F32, tag="o")
                for ts in range(4):
                    ps = psp.tile([128, OUT_F], F32, tag="ps")
                    for kt in range(KT):
                        nc.tensor.matmul(
                            ps[:],
                            xb[:, kt * 512 + ts * 128 : kt * 512 + ts * 128 + 128],
                            wmix[(g, kt)][:],
                            start=(kt == 0),
                            stop=False,
                        )
                    nc.tensor.matmul(ps[:], onest[:], mbt[g][:], start=False, stop=True)
                    nc.scalar.copy(oc[:, ts * OUT_F : (ts + 1) * OUT_F], ps[:])
                nc.sync.dma_start(
                    out[t0 : t0 + 512, :].rearrange("(ts p) o -> p ts o", p=128),
                    oc[:].rearrange("p (ts o) -> p ts o", ts=4),
                )
    nc.finalize()
    return nc


def kernel(x, coefficients, weight_experts, bias_experts, weight_shared, bias_shared, sizes):
    x = np.asarray(x)
    coefficients = np.asarray(coefficients)
    weight_experts = np.asarray(weight_experts)
    bias_experts = np.asarray(bias_experts)
    weight_shared = np.asarray(weight_shared)
    bias_shared = np.asarray(bias_shared)

    if "nc" not in _CACHE:
        _CACHE["nc"] = _build()
    nc = _CACHE["nc"]

    # ---- host-side layout prep (no arithmetic) ----
    wt_np = np.empty((N_EXPERTS + 1, IN_F, OUT_F), np.float32)
    for e in range(N_EXPERTS):
        wt_np[e] = weight_experts[e].T
    wt_np[N_EXPERTS] = weight_shared.T
    ball_np = np.empty((N_EXPERTS + 1, OUT_F), np.float32)
    ball_np[:N_EXPERTS] = bias_experts
    ball_np[N_EXPERTS] = bias_shared
    ones_np = np.ones((1, 128), ml_dtypes.bfloat16)

    in_maps = []
    for c in range(N_CORES):
        gs = slice(c * G_PER_CORE, (c + 1) * G_PER_CORE)
        cg = coefficients[gs]  # [4, 8]
        cb_np = np.broadcast_to(
            cg.reshape(1, -1), (128, G_PER_CORE * N_EXPERTS)
        ).copy()
        cx_np = np.empty((N_EXPERTS + 1, G_PER_CORE), np.float32)
        cx_np[:N_EXPERTS] = cg.T
        cx_np[N_EXPERTS] = 1.0
        xT_np = np.ascontiguousarray(
            x[c * TOK_PER_CORE : (c + 1) * TOK_PER_CORE].T
        )
        in_maps.append(
            {
                "xT": xT_np,
                "wt": wt_np,
                "cb": cb_np,
                "cx": cx_np,
                "ball": ball_np,
                "ones": ones_np,
            }
        )

    res = run_bass_kernel_spmd(nc, in_maps, core_ids=list(range(N_CORES)))
    return np.concatenate([res.results[c]["out"] for c in range(N_CORES)], axis=0)



# revision 4
# speedup vs baseline: 1.9070x; 1.9070x over previous
"""MOLELinear (mixture-of-linear-experts) Trainium2 kernel.

Math (per group g): out_g = x_g @ (sum_e c[g,e] W_e + W_sh).T + (sum_e c[g,e] b_e + b_sh)

Sharding: data-parallel over the 32 groups -> 4 groups (8192 tokens) per core,
expert weights replicated. Host does layout-only prep (transpose / stacking /
fp16 rounding); all arithmetic of the reference runs on device.

Device plan per core (all fp16 data path, fp32 accumulation in PSUM):
  - DMA in: xT [512, 8192] fp16 (x shard transposed), wall [128, 9, 2048] fp16
    (shared + 8 experts, k-tile-major free layout), small coefficient/bias
    tensors, cdiag [128, 9, 128] (scaled identity matrices for group 0).
  - Group-0 weight mix on PE during the weight-DMA window:
    psum[:, kt] += diag(c_j) @ W_j[kt]  (keeps PE warm, fp32 accumulation).
  - Groups 1-3 weight mix on DVE: fp16 scalar_tensor_tensor FMA chains,
    FD=2048 per op (8 ops per group).
  - Mixed biases transposed on PE: mbT[ot][o,g] = sum_j ball[j,o] cx[j,g].
  - Main GEMM: stationary = mixed weight subtile [128k,128o], moving = xT
    slice [128k,512t]; psum [128 o, 2048 t] accumulates 4 k-tiles.
  - Drain on ScalarE via ACTIVATE(Identity, bias=mbT column): PSUM->SBUF fp16
    with the per-partition bias add fused in. DMA out on GpSimd (SWDGE) so
    output stores don't contend with the input DMA issue queue.
"""
import numpy as np

import concourse.bacc as bacc
import concourse.mybir as mybir
from concourse.alu_op_type import AluOpType
from concourse.tile import TileContext
from concourse.bass_utils import run_bass_kernel_spmd

N_CORES = 8
IN_F = 512
OUT_F = 512
N_EXPERTS = 8
N_GROUPS = 32
TOK_PER_GROUP = 2048
G_PER_CORE = N_GROUPS // N_CORES           # 4
TOK_PER_CORE = G_PER_CORE * TOK_PER_GROUP  # 8192
KT = IN_F // 128                           # 4 k-tiles
OT = OUT_F // 128                          # 4 out-feature tiles
NW = N_EXPERTS + 1                         # 9: shared weight first, then experts
F32 = mybir.dt.float32
F32R = mybir.dt.float32r
F16 = mybir.dt.float16
AF = mybir.ActivationFunctionType

_CACHE = {}


def _build():
    nc = bacc.Bacc(trn_type="TRN2")
    xT = nc.dram_tensor("xT", (IN_F, TOK_PER_CORE), F16, kind="ExternalInput")
    wall = nc.dram_tensor("wall", (128, NW, KT * OUT_F), F16, kind="ExternalInput")
    cdiag = nc.dram_tensor("cdiag", (128, NW, 128), F16, kind="ExternalInput")
    cb = nc.dram_tensor("cb", (128, G_PER_CORE * N_EXPERTS), F32, kind="ExternalInput")
    cx = nc.dram_tensor("cx", (NW, G_PER_CORE), F32R, kind="ExternalInput")
    ball = nc.dram_tensor("ball", (NW, OUT_F), F32R, kind="ExternalInput")
    outT = nc.dram_tensor("outT", (OUT_F, TOK_PER_CORE), F16, kind="ExternalOutput")

    with TileContext(nc) as tc:
        with (
            tc.tile_pool(name="smallp", bufs=1) as smallp,
            tc.tile_pool(name="wallp", bufs=1) as wallp,
            tc.tile_pool(name="wmp", bufs=1) as wmp,
            tc.tile_pool(name="xp", bufs=1) as xp,
            tc.tile_pool(name="ocp", bufs=4) as ocp,
        ):
            # ---- small DMAs first ----
            cdt = smallp.tile([128, NW * 128], F16, tag="cdt")
            nc.sync.dma_start(cdt[:].rearrange("p (e m) -> p e m", e=NW), cdiag[:])
            cbt = smallp.tile([128, G_PER_CORE * N_EXPERTS], F32, tag="cb")
            nc.sync.dma_start(cbt[:], cb[:])
            cxt = smallp.tile([NW, G_PER_CORE], F32R, tag="cx")
            nc.sync.dma_start(cxt[:], cx[:])
            ballt = smallp.tile([NW, OUT_F], F32R, tag="ball")
            nc.sync.dma_start(ballt[:], ball[:])

            # ---- expert weights: one DMA per expert slice (shared at j=0) ----
            walls = wallp.tile([128, NW * KT * OUT_F], F16, tag="walls")
            for j in range(NW):
                nc.sync.dma_start(
                    walls[:, j * KT * OUT_F : (j + 1) * KT * OUT_F], wall[:, j, :]
                )

            # ---- x: group 0 per-k-tile for early start, groups 1-3 whole ----
            xg = [
                xp.tile([128, KT * TOK_PER_GROUP], F16, tag=f"x{g}", name=f"x{g}")
                for g in range(G_PER_CORE)
            ]
            for kt in range(KT):
                nc.sync.dma_start(
                    xg[0][:, kt * TOK_PER_GROUP : (kt + 1) * TOK_PER_GROUP],
                    xT[kt * 128 : (kt + 1) * 128, 0:TOK_PER_GROUP],
                )
            for g in range(1, G_PER_CORE):
                nc.sync.dma_start(
                    xg[g][:].rearrange("p (kt t) -> p kt t", kt=KT),
                    xT[:, g * TOK_PER_GROUP : (g + 1) * TOK_PER_GROUP].rearrange(
                        "(kt p) t -> p kt t", p=128
                    ),
                )

            # ---- mixed biases transposed: mbT[ot][o,g] = sum_j ball[j,o]cx[j,g]
            mbT = []
            with tc.tile_pool(name="psb", bufs=1, space="PSUM") as psb:
                for ot in range(OT):
                    pb = psb.tile([128, G_PER_CORE], F32, tag="pb")
                    nc.tensor.matmul(
                        pb[:],
                        ballt[:, ot * 128 : (ot + 1) * 128],
                        cxt[:],
                        start=True,
                        stop=True,
                    )
                    mb = smallp.tile([128, G_PER_CORE], F32, tag=f"mbT{ot}")
                    nc.vector.tensor_copy(mb[:], pb[:])
                    mbT.append(mb)

            wm = [
                wmp.tile([128, KT * OUT_F], F16, tag=f"wm{g}", name=f"wm{g}")
                for g in range(G_PER_CORE)
            ]

            with tc.tile_pool(name="ps", bufs=2, space="PSUM") as ps:
                # ---- group-0 weight mix on PE (runs while weights stream in)
                pm = ps.tile([128, KT * OUT_F], F32, tag="ps")
                for j in range(NW):
                    for kt in range(KT):
                        nc.tensor.matmul(
                            pm[:, kt * OUT_F : (kt + 1) * OUT_F],
                            cdt[:, j * 128 : (j + 1) * 128],
                            walls[:, (j * KT + kt) * OUT_F : (j * KT + kt + 1) * OUT_F],
                            start=(j == 0),
                            stop=(j == NW - 1),
                        )
                nc.scalar.copy(wm[0][:], pm[:])

                # ---- groups 1-3 weight mix on DVE: fp16 FMA chains ----
                for g in range(1, G_PER_CORE):
                    nc.vector.scalar_tensor_tensor(
                        wm[g][:],
                        walls[:, KT * OUT_F : 2 * KT * OUT_F],
                        cbt[:, g * N_EXPERTS : g * N_EXPERTS + 1],
                        walls[:, 0 : KT * OUT_F],
                        AluOpType.mult,
                        AluOpType.add,
                    )
                    for e in range(1, N_EXPERTS):
                        nc.vector.scalar_tensor_tensor(
                            wm[g][:],
                            walls[:, (e + 1) * KT * OUT_F : (e + 2) * KT * OUT_F],
                            cbt[:, g * N_EXPERTS + e : g * N_EXPERTS + e + 1],
                            wm[g][:],
                            AluOpType.mult,
                            AluOpType.add,
                        )

                # ---- main GEMM ----
                for g in range(G_PER_CORE):
                    for ot in range(OT):
                        pt = ps.tile([128, TOK_PER_GROUP], F32, tag="ps")
                        for kt in range(KT):
                            lhsT = wm[g][
                                :, kt * OUT_F + ot * 128 : kt * OUT_F + (ot + 1) * 128
                            ]
                            for tci in range(TOK_PER_GROUP // 512):
                                nc.tensor.matmul(
                                    pt[:, tci * 512 : (tci + 1) * 512],
                                    lhsT,
                                    xg[g][
                                        :,
                                        kt * TOK_PER_GROUP
                                        + tci * 512 : kt * TOK_PER_GROUP
                                        + (tci + 1) * 512,
                                    ],
                                    start=(kt == 0),
                                    stop=(kt == KT - 1),
                                )
                        oc = ocp.tile([128, TOK_PER_GROUP], F16, tag="oc")
                        nc.scalar.activation(
                            oc[:],
                            pt[:],
                            AF.Identity,
                            bias=mbT[ot][:, g : g + 1],
                            scale=1.0,
                        )
                        nc.gpsimd.dma_start(
                            outT[
                                ot * 128 : (ot + 1) * 128,
                                g * TOK_PER_GROUP : (g + 1) * TOK_PER_GROUP,
                            ],
                            oc[:],
                        )
    nc.finalize()
    return nc


def kernel(x, coefficients, weight_experts, bias_experts, weight_shared, bias_shared, sizes):
    x = np.asarray(x)
    coefficients = np.asarray(coefficients, dtype=np.float32)
    weight_experts = np.asarray(weight_experts, dtype=np.float32)
    bias_experts = np.asarray(bias_experts, dtype=np.float32)
    weight_shared = np.asarray(weight_shared, dtype=np.float32)
    bias_shared = np.asarray(bias_shared, dtype=np.float32)

    if "nc" not in _CACHE:
        _CACHE["nc"] = _build()
    nc = _CACHE["nc"]

    # ---- host-side layout prep ----
    x16 = x.astype(np.float16)
    # wall[p, j, kt*512+o] = W_j^T[kt*128+p, o]; j=0 shared, j=1+e expert e
    wall_np = np.empty((128, NW, KT * OUT_F), np.float16)
    for j in range(NW):
        W = weight_shared if j == 0 else weight_experts[j - 1]
        arr = W.T.reshape(KT, 128, OUT_F).transpose(1, 0, 2).reshape(128, KT * OUT_F)
        wall_np[:, j, :] = arr.astype(np.float16)
    ball_np = np.empty((NW, OUT_F), np.float32)
    ball_np[0] = bias_shared
    ball_np[1:] = bias_experts

    in_maps = []
    for c in range(N_CORES):
        gs = slice(c * G_PER_CORE, (c + 1) * G_PER_CORE)
        cg = coefficients[gs]  # [4, 8]
        cb_np = np.broadcast_to(
            cg.reshape(1, -1), (128, G_PER_CORE * N_EXPERTS)
        ).copy()
        cx_np = np.empty((NW, G_PER_CORE), np.float32)
        cx_np[0] = 1.0
        cx_np[1:] = cg.T
        cd_np = np.zeros((128, NW, 128), np.float16)
        idx = np.arange(128)
        cd_np[idx, 0, idx] = 1.0
        for e in range(N_EXPERTS):
            cd_np[idx, 1 + e, idx] = np.float16(cg[0, e])
        xT_np = np.ascontiguousarray(
            x16[c * TOK_PER_CORE : (c + 1) * TOK_PER_CORE].T
        )
        in_maps.append(
            {
                "xT": xT_np,
                "wall": wall_np,
                "cdiag": cd_np,
                "cb": cb_np,
                "cx": cx_np,
                "ball": ball_np,
            }
        )

    res = run_bass_kernel_spmd(nc, in_maps, core_ids=list(range(N_CORES)))
    out = np.empty((N_CORES * TOK_PER_CORE, OUT_F), np.float32)
    for c in range(N_CORES):
        out[c * TOK_PER_CORE : (c + 1) * TOK_PER_CORE] = (
            np.asarray(res.results[c]["outT"]).T.astype(np.float32)
        )
    return out


# revision 8
# speedup vs baseline: 2.1432x; 1.1239x over previous
"""MOLELinear (mixture-of-linear-experts) Trainium2 kernel.

Math (per group g): out_g = x_g @ (sum_e c[g,e] W_e + W_sh).T + (sum_e c[g,e] b_e + b_sh)

Sharding: data-parallel over the 32 groups -> 4 groups (8192 tokens) per core,
expert weights replicated. Host does layout-only prep (transpose / stacking /
fp16 rounding); all arithmetic of the reference runs on device.

Device plan per core (all fp16 data path, fp32 accumulation in PSUM):
  - DMA in: xT [512, 8192] fp16 (x shard transposed), wall [128, 9, 2048] fp16
    (shared + 8 experts, k-tile-major free layout), small coefficient/bias
    tensors, cdiag [128, 9, 128] (scaled identity matrices for group 0).
  - Group-0 weight mix on PE during the weight-DMA window:
    psum[:, kt] += diag(c_j) @ W_j[kt]  (keeps PE warm, fp32 accumulation).
  - Groups 1-3 weight mix on DVE: fp16 scalar_tensor_tensor FMA chains,
    FD=2048 per op (8 ops per group).
  - Mixed biases transposed on PE: mbT[ot][o,g] = sum_j ball[j,o] cx[j,g].
  - Main GEMM: stationary = mixed weight subtile [128k,128o], moving = xT
    slice [128k,512t]; psum [128 o, 2048 t] accumulates 4 k-tiles.
  - Drain on ScalarE via ACTIVATE(Identity, bias=mbT column): PSUM->SBUF fp16
    with the per-partition bias add fused in. DMA out on GpSimd (SWDGE) so
    output stores don't contend with the input DMA issue queue.
"""
import numpy as np

import concourse.bacc as bacc
import concourse.mybir as mybir
from concourse.alu_op_type import AluOpType
from concourse.tile import TileContext
from concourse.bass_utils import run_bass_kernel_spmd

N_CORES = 8
IN_F = 512
OUT_F = 512
N_EXPERTS = 8
N_GROUPS = 32
TOK_PER_GROUP = 2048
G_PER_CORE = N_GROUPS // N_CORES           # 4
TOK_PER_CORE = G_PER_CORE * TOK_PER_GROUP  # 8192
KT = IN_F // 128                           # 4 k-tiles
OT = OUT_F // 128                          # 4 out-feature tiles
NW = N_EXPERTS + 1                         # 9: shared weight first, then experts
F32 = mybir.dt.float32
F32R = mybir.dt.float32r
F16 = mybir.dt.float16
AF = mybir.ActivationFunctionType

_CACHE = {}


def _build():
    nc = bacc.Bacc(trn_type="TRN2")
    xT = nc.dram_tensor("xT", (IN_F, TOK_PER_CORE), F16, kind="ExternalInput")
    wall = nc.dram_tensor("wall", (128, NW, KT * OUT_F), F16, kind="ExternalInput")
    cdiag = nc.dram_tensor("cdiag", (128, NW, 128), F16, kind="ExternalInput")
    cb = nc.dram_tensor("cb", (128, G_PER_CORE * N_EXPERTS), F32, kind="ExternalInput")
    cx = nc.dram_tensor("cx", (NW, G_PER_CORE), F32R, kind="ExternalInput")
    ball = nc.dram_tensor("ball", (NW, OUT_F), F32R, kind="ExternalInput")
    outT = nc.dram_tensor("outT", (OUT_F, TOK_PER_CORE), F16, kind="ExternalOutput")

    with TileContext(nc) as tc:
        with (
            tc.tile_pool(name="smallp", bufs=1) as smallp,
            tc.tile_pool(name="wallp", bufs=1) as wallp,
            tc.tile_pool(name="wmp", bufs=1) as wmp,
            tc.tile_pool(name="xp", bufs=1) as xp,
            tc.tile_pool(name="ocp", bufs=4) as ocp,
        ):
            # ---- small DMAs first (bias inputs before everything) ----
            cxt = smallp.tile([NW, G_PER_CORE], F32R, tag="cx")
            nc.sync.dma_start(cxt[:], cx[:])
            ballt = smallp.tile([NW, OUT_F], F32R, tag="ball")
            nc.sync.dma_start(ballt[:], ball[:])
            cdt = smallp.tile([128, NW * 128], F16, tag="cdt")
            nc.sync.dma_start(cdt[:].rearrange("p (e m) -> p e m", e=NW), cdiag[:])

            # ---- expert weights: one DMA per expert slice (shared at j=0) ----
            walls = wallp.tile([128, NW * KT * OUT_F], F16, tag="walls")
            cbt = smallp.tile([128, G_PER_CORE * N_EXPERTS], F32, tag="cb")
            for j in range(NW):
                nc.sync.dma_start(
                    walls[:, j * KT * OUT_F : (j + 1) * KT * OUT_F], wall[:, j, :]
                )
                if j == 1:
                    # STT chains need the coefficient broadcast once expert 0 is in
                    nc.sync.dma_start(cbt[:], cb[:])

            # ---- x: group 0 per-k-tile for early start, groups 1-3 whole ----
            xg = [
                xp.tile([128, KT * TOK_PER_GROUP], F16, tag=f"x{g}", name=f"x{g}")
                for g in range(G_PER_CORE)
            ]
            for kt in range(KT):
                nc.sync.dma_start(
                    xg[0][:, kt * TOK_PER_GROUP : (kt + 1) * TOK_PER_GROUP],
                    xT[kt * 128 : (kt + 1) * 128, 0:TOK_PER_GROUP],
                )
            for g in range(1, G_PER_CORE):
                nc.sync.dma_start(
                    xg[g][:].rearrange("p (kt t) -> p kt t", kt=KT),
                    xT[:, g * TOK_PER_GROUP : (g + 1) * TOK_PER_GROUP].rearrange(
                        "(kt p) t -> p kt t", p=128
                    ),
                )

            wm = [
                wmp.tile([128, KT * OUT_F], F16, tag=f"wm{g}", name=f"wm{g}")
                for g in range(G_PER_CORE)
            ]

            with tc.tile_pool(name="ps", bufs=2, space="PSUM") as ps:
                # ---- mixed biases (one bank, freed early):
                # mbT2[o', ot*4+g] = sum_j ball[j, ot*128+o'] cx[j, g]
                pb = ps.tile([128, OT * G_PER_CORE], F32, tag="ps")
                for ot in range(OT):
                    nc.tensor.matmul(
                        pb[:, ot * G_PER_CORE : (ot + 1) * G_PER_CORE],
                        ballt[:, ot * 128 : (ot + 1) * 128],
                        cxt[:],
                        start=True,
                        stop=True,
                    )
                mbT2 = smallp.tile([128, OT * G_PER_CORE], F32, tag="mbT2")
                nc.scalar.copy(mbT2[:], pb[:])

                # ---- groups 1-3 weight mix on DVE, two-step per term:
                # tensor_scalar (may reach 4x mode) then tensor_tensor (2x)
                for g in (1, 2, 3):
                    for e in range(N_EXPERTS):
                        tmp = wmp.tile(
                            [128, KT * OUT_F], F16, tag="tmp", name="tmp", bufs=2
                        )
                        nc.vector.tensor_scalar(
                            tmp[:],
                            walls[:, (e + 1) * KT * OUT_F : (e + 2) * KT * OUT_F],
                            cbt[:, g * N_EXPERTS + e : g * N_EXPERTS + e + 1],
                            None,
                            AluOpType.mult,
                        )
                        nc.vector.tensor_tensor(
                            wm[g][:],
                            tmp[:],
                            walls[:, 0 : KT * OUT_F] if e == 0 else wm[g][:],
                            AluOpType.add,
                        )

                # ---- group-0 weight mix on PE (runs while weights stream in)
                pm = ps.tile([128, KT * OUT_F], F32, tag="ps")
                for j in range(NW):
                    for kt in range(KT):
                        nc.tensor.matmul(
                            pm[:, kt * OUT_F : (kt + 1) * OUT_F],
                            cdt[:, j * 128 : (j + 1) * 128],
                            walls[:, (j * KT + kt) * OUT_F : (j * KT + kt + 1) * OUT_F],
                            start=(j == 0),
                            stop=(j == NW - 1),
                        )
                nc.scalar.copy(wm[0][:], pm[:])

                # ---- main GEMM ----
                for g in range(G_PER_CORE):
                    for ot in range(OT):
                        pt = ps.tile([128, TOK_PER_GROUP], F32, tag="ps")
                        for kt in range(KT):
                            lhsT = wm[g][
                                :, kt * OUT_F + ot * 128 : kt * OUT_F + (ot + 1) * 128
                            ]
                            for tci in range(TOK_PER_GROUP // 512):
                                nc.tensor.matmul(
                                    pt[:, tci * 512 : (tci + 1) * 512],
                                    lhsT,
                                    xg[g][
                                        :,
                                        kt * TOK_PER_GROUP
                                        + tci * 512 : kt * TOK_PER_GROUP
                                        + (tci + 1) * 512,
                                    ],
                                    start=(kt == 0),
                                    stop=(kt == KT - 1),
                                )
                        oc = ocp.tile([128, TOK_PER_GROUP], F16, tag="oc")
                        nc.scalar.activation(
                            oc[:],
                            pt[:],
                            AF.Identity,
                            bias=mbT2[:, ot * G_PER_CORE + g : ot * G_PER_CORE + g + 1],
                            scale=1.0,
                        )
                        nc.scalar.dma_start(
                            outT[
                                ot * 128 : (ot + 1) * 128,
                                g * TOK_PER_GROUP : (g + 1) * TOK_PER_GROUP,
                            ],
                            oc[:],
                        )
    nc.finalize()
    return nc


def kernel(x, coefficients, weight_experts, bias_experts, weight_shared, bias_shared, sizes):
    x = np.asarray(x)
    coefficients = np.asarray(coefficients, dtype=np.float32)
    weight_experts = np.asarray(weight_experts, dtype=np.float32)
    bias_experts = np.asarray(bias_experts, dtype=np.float32)
    weight_shared = np.asarray(weight_shared, dtype=np.float32)
    bias_shared = np.asarray(bias_shared, dtype=np.float32)

    if "nc" not in _CACHE:
        _CACHE["nc"] = _build()
    nc = _CACHE["nc"]

    # ---- host-side layout prep ----
    x16 = x.astype(np.float16)
    # wall[p, j, kt*512+o] = W_j^T[kt*128+p, o]; j=0 shared, j=1+e expert e
    wall_np = np.empty((128, NW, KT * OUT_F), np.float16)
    for j in range(NW):
        W = weight_shared if j == 0 else weight_experts[j - 1]
        arr = W.T.reshape(KT, 128, OUT_F).transpose(1, 0, 2).reshape(128, KT * OUT_F)
        wall_np[:, j, :] = arr.astype(np.float16)
    ball_np = np.empty((NW, OUT_F), np.float32)
    ball_np[0] = bias_shared
    ball_np[1:] = bias_experts

    in_maps = []
    for c in range(N_CORES):
        gs = slice(c * G_PER_CORE, (c + 1) * G_PER_CORE)
        cg = coefficients[gs]  # [4, 8]
        cb_np = np.broadcast_to(
            cg.reshape(1, -1), (128, G_PER_CORE * N_EXPERTS)
        ).copy()
        cx_np = np.empty((NW, G_PER_CORE), np.float32)
        cx_np[0] = 1.0
        cx_np[1:] = cg.T
        cd_np = np.zeros((128, NW, 128), np.float16)
        idx = np.arange(128)
        cd_np[idx, 0, idx] = 1.0
        for e in range(N_EXPERTS):
            cd_np[idx, 1 + e, idx] = np.float16(cg[0, e])
        xT_np = np.ascontiguousarray(
            x16[c * TOK_PER_CORE : (c + 1) * TOK_PER_CORE].T
        )
        in_maps.append(
            {
                "xT": xT_np,
                "wall": wall_np,
                "cdiag": cd_np,
                "cb": cb_np,
                "cx": cx_np,
                "ball": ball_np,
            }
        )

    res = run_bass_kernel_spmd(nc, in_maps, core_ids=list(range(N_CORES)))
    out = np.empty((N_CORES * TOK_PER_CORE, OUT_F), np.float32)
    for c in range(N_CORES):
        out[c * TOK_PER_CORE : (c + 1) * TOK_PER_CORE] = (
            np.asarray(res.results[c]["outT"]).T.astype(np.float32)
        )
    return out
